# revision 14
# baseline (speedup 1.0000x reference)
"""GAT (2-layer graph attention network) on 8 Trainium2 NeuronCores.

Strategy: node partition. Core c owns nodes [c*6250, (c+1)*6250) and all edges
whose src lies in its range (segment sums in the reference are over src).
Host-side preprocessing (index manipulation only): sort edges by src, group by
128-node src tile, split each tile's edges into two classes by dst table half
(int16 gather indices), pad each class to a fixed chunk count, and build fp8
one-hot mask tensors.

Per layer each core computes node features for its own nodes (dense matmuls)
into a 256B-row table (f16 payload in f32-declared rows), AllGathers it, then
runs the sparse phase in groups of GT src tiles: per group TWO dma_gather
calls (single_packet=False) fetch all [h|er] rows by dst — one per table
half, since dma_gather indices are int16. Attention coefficients use a
host-precomputed transposed fp8 mask (el expansion via PE matmul), and
segment sums are one-hot fp8-mask matmuls accumulated in PSUM. This replaces
the baseline's per-chunk indirect DMAs whose ~1us SWDGE fixed cost per
128-edge chunk dominated the runtime.

Self-contained: only needs numpy + the concourse (Bass) stack at
/opt/trn_rl_repo. All shapes hardcoded for the nn_GAT problem.
"""
import sys

if "/opt/trn_rl_repo" not in sys.path:
    sys.path.insert(0, "/opt/trn_rl_repo")

import ml_dtypes
import numpy as np

import concourse.bacc as bacc
import concourse.bass as bass
import concourse.mybir as mybir
import concourse.tile as tile
from concourse.bass_utils import run_bass_kernel_spmd
from concourse.masks import make_identity

# problem shapes
N = 50000
E = 800000
FIN = 256
H = 8          # heads, layer 1
F1 = 32        # per-head features, layer 1
NH = 256       # hidden = H*F1
C = 47         # classes
NCORES = 8
NPC = N // NCORES          # nodes per core = 6250
T = (NPC + 127) // 128     # src tiles per core = 49
LAST_ROWS = NPC - (T - 1) * 128   # rows in last tile = 106
NPAD = T * 128             # padded node count per core = 6272
GT = 7                     # src tiles per gather group (T = 7*7)
NG = T // GT               # groups per layer
HALF = 25088               # balanced dst-class split (int16-safe)
RT_ = NCORES * NPAD        # global padded table rows = 50176

F32 = mybir.dt.float32
F16 = mybir.dt.float16
F8 = mybir.dt.float8e4
I16 = mybir.dt.int16

ALU = mybir.AluOpType
ACT = mybir.ActivationFunctionType

_cache = {}


def _idx16(stream):
    """[n] int stream -> [128, n/16] int16 (16-wrap, replicated x8)."""
    n = len(stream)
    a = np.zeros((16, n // 16), np.int16)
    a[np.arange(n) % 16, np.arange(n) // 16] = stream.astype(np.int16)
    return np.tile(a, (8, 1))


def _preprocess(x, edge_src, edge_dst):
    """Sort/group edges; build per-core gather streams and fp8 masks."""
    order = np.argsort(edge_src, kind="stable")
    src_s = np.asarray(edge_src)[order].astype(np.int64)
    dst_s = np.asarray(edge_dst)[order].astype(np.int64)
    dstp_s = (dst_s // NPC) * NPAD + (dst_s % NPC)   # padded table row
    bounds = np.searchsorted(src_s, np.arange(NCORES + 1) * NPC)

    # per-(core,tile) class-A/B edge lists; global max chunk counts
    KA = KB = 1
    percore = []
    for c in range(NCORES):
        b0, b1 = bounds[c], bounds[c + 1]
        sc = (src_s[b0:b1] - c * NPC).astype(np.int64)
        dp = dstp_s[b0:b1]
        tloc = sc >> 7
        tiles = []
        for t in range(T):
            m = tloc == t
            st, dt_ = sc[m] - t * 128, dp[m]
            a = dt_ < HALF
            tiles.append((st[a], dt_[a], st[~a], dt_[~a] - HALF))
            KA = max(KA, (len(dt_[a]) + 127) // 128)
            KB = max(KB, (len(dt_[~a]) + 127) // 128)
        percore.append(tiles)

    CPT2 = KA + KB
    ins = []
    for c in range(NCORES):
        tiles = percore[c]
        srcloc = np.full((T, 128, CPT2), 1000.0, dtype=np.float16)
        iA = np.zeros((T, KA, 128), np.int64)
        iB = np.zeros((T, KB, 128), np.int64)
        for t in range(T):
            sA, dA, sB, dB = tiles[t]
            for (ss, dd, K0, koff, idst) in ((sA, dA, KA, 0, iA),
                                             (sB, dB, KB, KA, iB)):
                nn = len(ss)
                j = np.arange(nn)
                k, p = j // 128, j % 128
                srcloc[t, p, koff + k] = ss.astype(np.float16)
                idst[t, k, p] = dd
        # gather streams per group: [NG][GT*K*128] -> int16 wrap per group
        sa = np.concatenate([_idx16(iA[g * GT:(g + 1) * GT].ravel())
                             for g in range(NG)], axis=1)
        sb = np.concatenate([_idx16(iB[g * GT:(g + 1) * GT].ravel())
                             for g in range(NG)], axis=1)
        # fp8 agg mask: msk[t, p_edge, k, j] = (srcloc[t, p, k] == j)
        mskh = (srcloc[:, :, :, None] ==
                np.arange(128, dtype=np.float16)[None, None, None, :])
        mskh = np.ascontiguousarray(
            mskh.transpose(1, 0, 2, 3).reshape(128, -1)
        ).astype(ml_dtypes.float8_e4m3)
        # fp8 transposed mask: mskT[t, p, k*128+j] = (srcloc[t, j, k] == p)
        jj = srcloc.transpose(0, 2, 1)           # [T, CPT2, 128] edge src vals
        mskT = (jj[:, :, :, None] ==
                np.arange(128, dtype=np.float16)[None, None, None, :])
        # partition-major: [128, T*CPT2*128]
        mskT = np.ascontiguousarray(
            mskT.transpose(3, 0, 1, 2).reshape(128, -1)
        ).astype(ml_dtypes.float8_e4m3)
        ins.append({
            "srcg": np.ascontiguousarray(
                srcloc.transpose(1, 0, 2)).reshape(128, T * CPT2),
            "iA": sa, "iB": sb,
            "mskT": mskT, "msk": mskh,
        })
    return ins, KA, KB


def _build(KA, KB, debug=False):
    CPT2 = KA + KB
    nc = bacc.Bacc("TRN2", target_bir_lowering=False, debug=False,
                   num_devices=NCORES, num_swdge_queues=4)

    # ---- external inputs (per core) ----
    d_xT = nc.dram_tensor("xT", [2, 128, NPAD], F32, kind="ExternalInput")
    d_W1 = nc.dram_tensor("W1", [2, 128, F1], F32, kind="ExternalInput")
    d_W1T = nc.dram_tensor("W1T", [F1, FIN], F32, kind="ExternalInput")
    d_Wl1 = nc.dram_tensor("Wl1", [F1, H], F32, kind="ExternalInput")
    d_Wr1 = nc.dram_tensor("Wr1", [F1, H], F32, kind="ExternalInput")
    d_W2 = nc.dram_tensor("W2", [2, 128, C], F16, kind="ExternalInput")
    d_W2T = nc.dram_tensor("W2T", [C, NH], F32, kind="ExternalInput")
    d_Wl2 = nc.dram_tensor("Wl2", [C, 1], F32, kind="ExternalInput")
    d_Wr2 = nc.dram_tensor("Wr2", [C, 1], F32, kind="ExternalInput")
    d_b1 = nc.dram_tensor("b1f", [128, NH], F32, kind="ExternalInput")
    d_b2 = nc.dram_tensor("b2f", [128, C], F32, kind="ExternalInput")
    d_iota = nc.dram_tensor("iota", [128, 128], F16, kind="ExternalInput")
    d_srcg = nc.dram_tensor("srcg", [128, T * CPT2], F16, kind="ExternalInput")
    d_iA = nc.dram_tensor("iA", [128, T * KA * 8], I16, kind="ExternalInput")
    d_iB = nc.dram_tensor("iB", [128, T * KB * 8], I16, kind="ExternalInput")
    d_mskT = nc.dram_tensor("mskT", [128, T * CPT2 * 128], F8,
                            kind="ExternalInput")
    d_msk = nc.dram_tensor("msk", [128, T * CPT2 * 128], F8,
                           kind="ExternalInput")

    d_out = nc.dram_tensor("out", [NPC, C], F32, kind="ExternalOutput")

    # ---- internal DRAM tables: 256B rows (f16 payload via bitcast) ----
    d_t1loc = nc.dram_tensor("t1loc", [NPAD, 20], F32)
    d_el1 = nc.dram_tensor("el1", [NPAD, H], F16)
    d_t1c = nc.dram_tensor("t1c", [RT_, 20], F32, addr_space="Shared")
    d_t1 = nc.dram_tensor("t1", [RT_, 64], F32)
    d_t2loc = nc.dram_tensor("t2loc", [NPAD, 24], F32)
    d_el2 = nc.dram_tensor("el2", [NPAD, 1], F16)
    d_t2c = nc.dram_tensor("t2c", [RT_, 24], F32, addr_space="Shared")
    d_t2 = nc.dram_tensor("t2", [RT_, 64], F32)

    groups = [list(range(NCORES))]

    with tile.TileContext(nc, num_cores=NCORES) as tc:
        with (
            tc.tile_pool(name="const", bufs=1) as cpool,
            tc.tile_pool(name="rt", bufs=1) as rtpool,
            tc.tile_pool(name="gat", bufs=2) as gp,     # gathered tables
            tc.tile_pool(name="mk", bufs=1) as mkp,     # masks
            tc.tile_pool(name="work", bufs=2) as wp,
            tc.tile_pool(name="small", bufs=3) as sp,
            tc.tile_pool(name="psA", bufs=2, space="PSUM") as psA,
            tc.tile_pool(name="psD", bufs=2, space="PSUM") as psD,
            tc.tile_pool(name="psT", bufs=2, space="PSUM") as psT,
            tc.tile_pool(name="psE", bufs=2, space="PSUM") as psE,
        ):
            # ---------- constants ----------
            b1sb = cpool.tile([128, NH], F32)
            nc.sync.dma_start(out=b1sb[:], in_=d_b1.ap())
            b2sb = cpool.tile([128, C], F32)
            nc.sync.dma_start(out=b2sb[:], in_=d_b2.ap())
            ident = cpool.tile([128, 128], F16)
            make_identity(nc, ident[:])
            W1cat = cpool.tile([128, 2, 48], F32)
            nc.sync.dma_start(out=W1cat[:, :, 0:32],
                              in_=d_W1.ap().rearrange("q p f -> p q f"))
            W2cat = cpool.tile([128, 2, 49], F16)
            nc.sync.dma_start(out=W2cat[:, :, 0:47],
                              in_=d_W2.ap().rearrange("q p f -> p q f"))
            W1Tsb = cpool.tile([F1, FIN], F32)
            nc.sync.dma_start(out=W1Tsb[:], in_=d_W1T.ap())
            W2Tsb = cpool.tile([C, NH], F32)
            nc.sync.dma_start(out=W2Tsb[:], in_=d_W2T.ap())
            Wl1sb = cpool.tile([F1, H], F32)
            nc.sync.dma_start(out=Wl1sb[:], in_=d_Wl1.ap())
            Wr1sb = cpool.tile([F1, H], F32)
            nc.sync.dma_start(out=Wr1sb[:], in_=d_Wr1.ap())
            Wl2sb = cpool.tile([C, 1], F32)
            nc.sync.dma_start(out=Wl2sb[:], in_=d_Wl2.ap())
            Wr2sb = cpool.tile([C, 1], F32)
            nc.sync.dma_start(out=Wr2sb[:], in_=d_Wr2.ap())
            iAsb = cpool.tile([128, T * KA * 8], I16)
            nc.sync.dma_start(out=iAsb[:], in_=d_iA.ap())
            iBsb = cpool.tile([128, T * KB * 8], I16)
            nc.sync.dma_start(out=iBsb[:], in_=d_iB.ap())

            # A1/B1 = W1 @ Wl1 / W1 @ Wr1; A2/B2 = W2 @ Wl2 / W2 @ Wr2
            for q in range(2):
                pa = psD.tile([128, H], F32, tag="dense")
                nc.tensor.matmul(out=pa[:], lhsT=W1Tsb[:, q * 128:(q + 1) * 128],
                                 rhs=Wl1sb[:], start=True, stop=True)
                nc.vector.tensor_copy(out=W1cat[:, q, 40:48], in_=pa[:])
                pb = psD.tile([128, H], F32, tag="dense")
                nc.tensor.matmul(out=pb[:], lhsT=W1Tsb[:, q * 128:(q + 1) * 128],
                                 rhs=Wr1sb[:], start=True, stop=True)
                nc.vector.tensor_copy(out=W1cat[:, q, 32:40], in_=pb[:])
                pc = psD.tile([128, 1], F32, tag="dense")
                nc.tensor.matmul(out=pc[:], lhsT=W2Tsb[:, q * 128:(q + 1) * 128],
                                 rhs=Wl2sb[:], start=True, stop=True)
                nc.vector.tensor_copy(out=W2cat[:, q, 48:49], in_=pc[:])
                pd = psD.tile([128, 1], F32, tag="dense")
                nc.tensor.matmul(out=pd[:], lhsT=W2Tsb[:, q * 128:(q + 1) * 128],
                                 rhs=Wr2sb[:], start=True, stop=True)
                nc.vector.tensor_copy(out=W2cat[:, q, 47:48], in_=pd[:])

            rT = rtpool.tile([128, 2, NPAD], F16)

            # ---------- phase D1 ----------
            for t in range(T):
                xa = wp.tile([128, 2, 128], F32, tag="xa")
                nc.sync.dma_start(
                    out=xa[:], in_=d_xT.ap()[:, :, t * 128:(t + 1) * 128]
                        .rearrange("q p n -> p q n"))
                ps = psD.tile([128, 48], F32, tag="dense")
                for q in range(2):
                    nc.tensor.matmul(out=ps[:], lhsT=xa[:, q, :],
                                     rhs=W1cat[:, q, :], start=q == 0,
                                     stop=q == 1)
                hsb = wp.tile([128, 48], F16, tag="hsb")
                nc.scalar.copy(out=hsb[:], in_=ps[:])
                nc.sync.dma_start(
                    out=d_t1loc.ap().bitcast(F16)[t * 128:(t + 1) * 128, 0:40],
                    in_=hsb[:, 0:40])
                nc.sync.dma_start(out=d_el1.ap()[t * 128:(t + 1) * 128, :],
                                  in_=hsb[:, 40:48])

            # ---------- C1 (compact) + repack ----------
            nc.gpsimd.collective_compute(
                "AllGather", ALU.bypass, replica_groups=groups,
                ins=[d_t1loc.ap()], outs=[d_t1c.ap()])
            tc.strict_bb_all_engine_barrier()
            nc.sync.dma_start(out=d_t1.ap()[:, 0:20], in_=d_t1c.ap())
            tc.strict_bb_all_engine_barrier()

            def sparse_phase(d_tab, d_el, fdim, layer, after_group=None):
                """fdim: payload cols in f16 view (40 or 48); er at fdim-Hh."""
                Hh = H if layer == 1 else 1
                for g in range(NG):
                    if after_group and g in after_group:
                        after_group[g]()
                    GA = gp.tile([128, GT * KA, 64], F32, tag="GA")
                    ins_a = nc.gpsimd.dma_gather(
                        GA[:], d_tab.ap()[0:HALF, :],
                        iAsb[:, g * GT * KA * 8:(g + 1) * GT * KA * 8],
                        GT * KA * 128, GT * KA * 128, 64,
                        single_packet=False, queue_num=(2 * g) % 4)
                    GB = gp.tile([128, GT * KB, 64], F32, tag="GB")
                    ins_b = nc.gpsimd.dma_gather(
                        GB[:], d_tab.ap()[HALF:RT_, :],
                        iBsb[:, g * GT * KB * 8:(g + 1) * GT * KB * 8],
                        GT * KB * 128, GT * KB * 128, 64,
                        single_packet=False, queue_num=(2 * g + 1) % 4)
                    GAh = GA[:].bitcast(F16)   # [128, GT*KA, 128]
                    GBh = GB[:].bitcast(F16)
                    W = CPT2 * 128
                    mskTg = mkp.tile([128, GT * W], F8, tag="mskTg")
                    nc.sync.dma_start(
                        out=mskTg[:],
                        in_=d_mskT.ap()[:, g * GT * W:(g + 1) * GT * W])
                    mskg = mkp.tile([128, GT * W], F8, tag="mskg")
                    nc.sync.dma_start(
                        out=mskg[:],
                        in_=d_msk.ap()[:, g * GT * W:(g + 1) * GT * W])
                    for trel in range(GT):
                        t = g * GT + trel
                        mskT = mskTg[:, trel * W:(trel + 1) * W]
                        msk = mskg[:, trel * W:(trel + 1) * W] \
                            .rearrange("p (k j) -> p k j", j=128)
                        elt = sp.tile([128, Hh], F16, tag=f"elt{layer}")
                        nc.sync.dma_start(
                            out=elt[:], in_=d_el.ap()[t * 128:(t + 1) * 128, :])
                        pseT = psE.tile([128, CPT2 * H], F32, tag="elexp")
                        pse = pseT[:, 0:CPT2 * Hh]
                        for k in range(CPT2):
                            nc.tensor.matmul(
                                out=pse[:, k * Hh:(k + 1) * Hh],
                                lhsT=mskT[:, k * 128:(k + 1) * 128],
                                rhs=elt[:],
                                start=k == 0, stop=k == CPT2 - 1,
                                skip_group_check=True)

                        # s = er[dst] + el[src]; e = exp(leaky_relu(s))
                        FD = 264 if layer == 1 else 48
                        rhs = wp.tile([128, CPT2, FD], F16,
                                      tag=f"rhs{layer}")
                        s = sp.tile([128, CPT2, Hh], F32, tag=f"s{layer}")
                        era = GAh[:, trel * KA:(trel + 1) * KA,
                                  (fdim - Hh) * 1:fdim]
                        erb = GBh[:, trel * KB:(trel + 1) * KB,
                                  (fdim - Hh) * 1:fdim]
                        nc.vector.tensor_tensor(
                            out=s[:, 0:KA, :], in0=era,
                            in1=pse[:, 0:KA * Hh]
                                .rearrange("p (k h) -> p k h", h=Hh),
                            op=ALU.add)
                        nc.vector.tensor_tensor(
                            out=s[:, KA:CPT2, :], in0=erb,
                            in1=pse[:, KA * Hh:CPT2 * Hh]
                                .rearrange("p (k h) -> p k h", h=Hh),
                            op=ALU.add)
                        sL = sp.tile([128, CPT2, Hh], F16, tag=f"sL{layer}")
                        nc.vector.scalar_tensor_tensor(
                            out=sL[:], in0=s[:], scalar=0.2, in1=s[:],
                            op0=ALU.mult, op1=ALU.max)
                        nc.scalar.activation(out=rhs[:, :, FD - Hh:FD],
                                             in_=sL[:], func=ACT.Exp)
                        # msg = e * h[dst]
                        FF = F1 if layer == 1 else C
                        nc.vector.tensor_tensor(
                            out=rhs[:, 0:KA, 0:Hh * FF]
                                .rearrange("p k (h f) -> p k h f", h=Hh),
                            in0=rhs[:, 0:KA, FD - Hh:FD]
                                .rearrange("p k (h o) -> p k h o", o=1)
                                .to_broadcast([128, KA, Hh, FF]),
                            in1=GAh[:, trel * KA:(trel + 1) * KA, 0:fdim - Hh]
                                .rearrange("p k (o f) -> p k o f", o=1)
                                .to_broadcast([128, KA, Hh, FF]),
                            op=ALU.mult)
                        nc.vector.tensor_tensor(
                            out=rhs[:, KA:CPT2, 0:Hh * FF]
                                .rearrange("p k (h f) -> p k h f", h=Hh),
                            in0=rhs[:, KA:CPT2, FD - Hh:FD]
                                .rearrange("p k (h o) -> p k h o", o=1)
                                .to_broadcast([128, KB, Hh, FF]),
                            in1=GBh[:, trel * KB:(trel + 1) * KB, 0:fdim - Hh]
                                .rearrange("p k (o f) -> p k o f", o=1)
                                .to_broadcast([128, KB, Hh, FF]),
                            op=ALU.mult)

                        ps1T = psA.tile([128, 264], F32, tag="agg")
                        ps1 = ps1T[:, 0:FD]
                        for k in range(CPT2):
                            nc.tensor.matmul(out=ps1[:], lhsT=msk[:, k, :],
                                             rhs=rhs[:, k, :],
                                             start=k == 0, stop=k == CPT2 - 1)

                        if layer == 1:
                            _epilogue1(nc, wp, sp, psT, psD, ps1, b1sb, ident,
                                       rT, W2cat, t)
                        else:
                            _epilogue2(nc, wp, sp, ps1, b2sb, t)

            def _epilogue1(nc, wp, sp, psT, psD, ps1p, b1sb, ident, rT, W2cat, t):
                ps1 = wp.tile([128, 264], F32, tag="ps1s")
                nc.scalar.copy(out=ps1[:], in_=ps1p[:])
                dn = sp.tile([128, H], F32, tag="dn")
                nc.vector.tensor_scalar(out=dn[:], in0=ps1[:, 256:264],
                                        scalar1=1e-12, scalar2=None, op0=ALU.max)
                rc = sp.tile([128, H], F32, tag="rc")
                nc.vector.reciprocal(out=rc[:], in_=dn[:])
                o1 = wp.tile([128, NH], F32, tag="o1")
                nc.vector.tensor_tensor(
                    out=o1[:].rearrange("p (h f) -> p h f", h=H),
                    in0=ps1[:, 0:256].rearrange("p (h f) -> p h f", h=H),
                    in1=rc[:].rearrange("p (h o) -> p h o", o=1)
                        .to_broadcast([128, H, F1]),
                    op=ALU.mult)
                o1b = wp.tile([128, NH], F32, tag="o1b")
                nc.vector.tensor_tensor(out=o1b[:], in0=o1[:], in1=b1sb[:],
                                        op=ALU.add)
                p_ = wp.tile([128, NH], F32, tag="p_")
                nc.scalar.activation(out=p_[:], in_=o1b[:], func=ACT.Relu)
                q_ = wp.tile([128, NH], F32, tag="q_")
                nc.vector.tensor_scalar(out=q_[:], in0=o1b[:], scalar1=0.0,
                                        scalar2=None, op0=ALU.min)
                eq = wp.tile([128, NH], F32, tag="eq")
                nc.scalar.activation(out=eq[:], in_=q_[:], func=ACT.Exp)
                r_ = wp.tile([128, NH], F16, tag="r_")
                nc.vector.scalar_tensor_tensor(out=r_[:], in0=eq[:], scalar=-1.0,
                                               in1=p_[:], op0=ALU.add,
                                               op1=ALU.add)
                for q in range(2):
                    pt = psT.tile([128, 128], F16, tag="pt")
                    nc.tensor.transpose(out=pt[:],
                                        in_=r_[:, q * 128:(q + 1) * 128],
                                        identity=ident[:])
                    if q == 0:
                        nc.scalar.copy(out=rT[:, q, t * 128:(t + 1) * 128],
                                       in_=pt[:])
                    else:
                        nc.vector.tensor_copy(
                            out=rT[:, q, t * 128:(t + 1) * 128], in_=pt[:])
                # fused D2
                ps2 = psD.tile([128, 49], F32, tag="dense")
                for q in range(2):
                    nc.tensor.matmul(out=ps2[:],
                                     lhsT=rT[:, q, t * 128:(t + 1) * 128],
                                     rhs=W2cat[:, q, :], start=q == 0,
                                     stop=q == 1)
                h2sb = wp.tile([128, 49], F16, tag="h2sb")
                nc.scalar.copy(out=h2sb[:], in_=ps2[:])
                nc.sync.dma_start(
                    out=d_t2loc.ap().bitcast(F16)[t * 128:(t + 1) * 128, 0:48],
                    in_=h2sb[:, 0:48])
                nc.sync.dma_start(out=d_el2.ap()[t * 128:(t + 1) * 128, :],
                                  in_=h2sb[:, 48:49])

            def _epilogue2(nc, wp, sp, ps3p, b2sb, t):
                ps3 = wp.tile([128, 48], F32, tag="ps3s")
                nc.scalar.copy(out=ps3[:], in_=ps3p[:])
                dn2 = sp.tile([128, 1], F32, tag="dn2")
                nc.vector.tensor_scalar(out=dn2[:], in0=ps3[:, 47:48],
                                        scalar1=1e-12, scalar2=None,
                                        op0=ALU.max)
                rc2 = sp.tile([128, 1], F32, tag="rc2")
                nc.vector.reciprocal(out=rc2[:], in_=dn2[:])
                o2b = wp.tile([128, C], F32, tag="o2b")
                nc.vector.scalar_tensor_tensor(out=o2b[:], in0=ps3[:, 0:47],
                                               scalar=rc2[:, 0:1], in1=b2sb[:],
                                               op0=ALU.mult, op1=ALU.add)
                mx = sp.tile([128, 1], F32, tag="mx")
                nc.vector.tensor_reduce(out=mx[:], in_=o2b[:],
                                        axis=mybir.AxisListType.X, op=ALU.max)
                xm = wp.tile([128, C], F32, tag="xm")
                nc.vector.tensor_scalar(out=xm[:], in0=o2b[:],
                                        scalar1=mx[:, 0:1], scalar2=None,
                                        op0=ALU.subtract)
                ex = wp.tile([128, C], F32, tag="ex")
                se = sp.tile([128, 1], F32, tag="se")
                nc.scalar.activation(out=ex[:], in_=xm[:], func=ACT.Exp,
                                     accum_out=se[:])
                ls = sp.tile([128, 1], F32, tag="ls")
                nc.scalar.activation(out=ls[:], in_=se[:], func=ACT.Ln)
                fin = wp.tile([128, C], F32, tag="fin")
                nc.vector.tensor_scalar(out=fin[:], in0=xm[:],
                                        scalar1=ls[:, 0:1], scalar2=None,
                                        op0=ALU.subtract)
                rows = 128 if t < T - 1 else LAST_ROWS
                nc.sync.dma_start(out=d_out.ap()[t * 128:t * 128 + rows, :],
                                  in_=fin[0:rows, :])

            # ---------- S1 (+ fused D2) ----------
            sparse_phase(d_t1, d_el1, 40, 1)

            # ---------- C2 (compact) + repack into 256B-row gather table ----
            nc.gpsimd.collective_compute(
                "AllGather", ALU.bypass, replica_groups=groups,
                ins=[d_t2loc.ap()], outs=[d_t2c.ap()])
            tc.strict_bb_all_engine_barrier()
            nc.sync.dma_start(out=d_t2.ap()[:, 0:24], in_=d_t2c.ap())
            tc.strict_bb_all_engine_barrier()

            # ---------- S2 ----------
            sparse_phase(d_t2, d_el2, 48, 2)

    nc.compile()
    return nc


def _make_inputs(x, edge_src, edge_dst, W1, Wl1, Wr1, b1, W2, Wl2, Wr2, b2):
    edge_ins, KA, KB = _preprocess(x, edge_src, edge_dst)
    x = np.asarray(x, dtype=np.float32)
    W1 = np.asarray(W1, dtype=np.float32)
    W2 = np.asarray(W2, dtype=np.float32)
    iota = np.tile(np.arange(128, dtype=np.float16), (128, 1))
    b1f = np.tile(np.tile(np.asarray(b1, np.float32), H)[None, :], (128, 1))
    b2f = np.tile(np.asarray(b2, np.float32)[None, :], (128, 1))
    common = {
        "W1": W1.reshape(2, 128, F1),
        "W1T": np.ascontiguousarray(W1.T),
        "Wl1": np.asarray(Wl1, np.float32),
        "Wr1": np.asarray(Wr1, np.float32),
        "W2": W2.reshape(2, 128, C).astype(np.float16),
        "W2T": np.ascontiguousarray(W2.T),
        "Wl2": np.asarray(Wl2, np.float32),
        "Wr2": np.asarray(Wr2, np.float32),
        "b1f": b1f, "b2f": b2f, "iota": iota,
    }
    in_maps = []
    for c in range(NCORES):
        xT = np.zeros((2, 128, NPAD), dtype=np.float32)
        xs = np.ascontiguousarray(x[c * NPC:(c + 1) * NPC].T)
        xT[:, :, :NPC] = xs.reshape(2, 128, NPC)
        m = dict(common)
        m["xT"] = xT
        m.update(edge_ins[c])
        in_maps.append(m)
    return in_maps, KA, KB


def _run(inputs, trace=False, debug=False):
    in_maps, KA, KB = _make_inputs(**inputs)
    key = (KA, KB, debug)
    if key not in _cache:
        _cache[key] = _build(KA, KB, debug=debug)
    nc = _cache[key]
    bkr = run_bass_kernel_spmd(nc, in_maps, list(range(NCORES)), trace=trace)
    out = np.concatenate([bkr.results[c]["out"] for c in range(NCORES)], axis=0)
    return out.astype(np.float32), bkr


def kernel(**inputs):
    out, _ = _run(inputs, trace=False)
    return out
